# revision 132
# baseline (speedup 1.0000x reference)
"""Block-causal GQA attention layer on 8 Trainium2 NeuronCores.

Sharding: 8 cores = batch(2) x head-group(4). Core c handles batch b=c//4 and
head group g=c%4 (q heads 4g..4g+3, kv head g). W_attn is column-sharded by
head group, W_proj row-sharded; each core computes a partial [T, C] output
(bf16, upcast on host) and the host sums the 4 partials per batch element.

All matmuls run in bf16 (fp32 PSUM accumulate); rel err vs the fp32
reference is ~6e-3. x is pre-transposed on the host into a per-t-chunk
blocked layout so no PE transposes of x are needed, and the rope cos/sin
tables (with qk-norm weights and the 1/sqrt(d) softmax scale folded in) are
host-precomputed.

Per-core device pipeline:
  B) software-pipelined over 16 t-chunks: QKV matmuls (x-chunk stationary,
     W_attn streaming, fp32 PSUM) -> RMS stats (ACT square-accum) ->
     RoPE applied by DVE scalar_tensor_tensor ops reading raw q/k straight
     from PSUM, with the per-row rsqrt folded into the rope multiplies
     (rope commutes with a per-row scalar); rs_k instead rides as the
     per-partition exp scale in phase C. PE transposes (lag 2) produce
     qT/kT in [d, t] layout. The last chunk is staged through SBUF so all
     PSUM banks hand off to phase C without waiting on its rope, and its
     transposes are emitted after Ti=0.
  C) per 512-wide T-block Ti and head h: scores sT = kT.T @ qT on the exact
     128-granular block-causal staircase, exp on ACT (per-partition rs_k
     scale) into bf16, staircase mask multiply (DVE, bf16 2x) on diagonal
     tiles only, PV matmuls, and a denominator ones-matmul where groups of
     four full-width exp tiles are pre-summed on DVE so the PE streams each
     quad once. sc/exp run LAG=3 chunks ahead of PV; each head's final
     pv/den pair is preceded by the next head's warmup so exp latency never
     surfaces. approx-reciprocal normalize writes yT (bf16).
  D) output projection emitted as filler e-blocks interleaved into the
     (ACT-paced) attention S-stream, one per two S-chunks, with batched
     per-t-chunk output DMAs (bf16).
"""

import numpy as np

import concourse.bacc as bacc
import concourse.bass as bass
import concourse.tile as tile
import concourse.mybir as mybir
from concourse.bass_utils import run_bass_kernel_spmd
from concourse.masks import make_identity

P = 128
T = 2048
C = 2048
N_HEAD = 16
N_KV = 4
HD = 128          # head dim
HG = N_HEAD // N_KV  # heads per group = 4
BLOCK = 16
EPS = 1e-5
ROPE_BASE = 500000.0
QCOLS = HG * HD   # 512 q cols per core
JCOLS = QCOLS + 2 * HD  # 768 qkv cols per core
NT = T // P       # 16 t-chunks
NC16 = C // P     # 16 c-chunks
SCALE = 1.0 / float(np.sqrt(np.float32(HD)))

F32 = mybir.dt.float32
BF16 = mybir.dt.bfloat16
NPBF16 = mybir.dt.np(mybir.dt.bfloat16)
AF = mybir.ActivationFunctionType
ALU = mybir.AluOpType


def build_nc():
    nc = bacc.Bacc("TRN2", target_bir_lowering=False)

    # xt[i, c, ci, t] = x[i*128 + t, ci*128 + c]: per-t-chunk pre-transposed
    # blocked layout prepared on host; one contiguous [128, 2048] DMA per chunk.
    xt = nc.dram_tensor("xt", [NT, P, NC16, P], BF16, kind="ExternalInput")
    wa = nc.dram_tensor("wa", [C, JCOLS], BF16, kind="ExternalInput")
    wp = nc.dram_tensor("wp", [QCOLS, C], BF16, kind="ExternalInput")
    # rope tables packed [csq | snq | csk | snk] along the free axis
    tbl = nc.dram_tensor("tbl", [T, 4 * HD], F32, kind="ExternalInput")
    dm1 = nc.dram_tensor("dm1", [P, P], BF16, kind="ExternalInput")
    out = nc.dram_tensor("out", [T, C], BF16, kind="ExternalOutput")

    with tile.TileContext(nc) as tc:
        with tc.tile_pool(name="persist", bufs=1) as persist:
            ident_f = persist.tile([P, P], F32)
            make_identity(nc, ident_f)
            ident = persist.tile([P, P], BF16)
            nc.vector.tensor_copy(ident, ident_f)
            rsk_all = persist.tile([P, NT], F32)
            ones = persist.tile([P, P], BF16)
            nc.vector.memset(ones, 1.0)
            dm1_sb = persist.tile([P, P], BF16)
            eps_sb = persist.tile([P, 1], F32)
            nc.vector.memset(eps_sb, EPS)

            qT = persist.tile([P, HG, T], BF16)     # [d, h, t]
            kT = persist.tile([P, T], BF16)         # [d, t]
            v_sb = persist.tile([P, NT, HD], BF16)  # [s_in_chunk, s_chunk, d']
            yT = persist.tile([P, HG, T], BF16)     # [d', h, t]
            wp_sb = persist.tile([P, HG, C], BF16)

            # ---------------- Phase B (software-pipelined) ---------------
            # Explicit pool lifetimes (released in stack order per side):
            # phase B PSUM = tp2 + qa3 + qb3 = 8 banks; released before the
            # staged chunk-15 tail so phase C's pools (sc4 + yt2/den2) can
            # claim all 8 banks with no handoff stall.
            if True:
                wts = tc.alloc_tile_pool(name="wts", bufs=1)
                bstream = tc.alloc_tile_pool(name="bstream", bufs=3)
                bwork = tc.alloc_tile_pool(name="bwork", bufs=4)
                psB_tp = tc.alloc_tile_pool(name="psB_tp", bufs=2, space="PSUM")
                psB_qa = tc.alloc_tile_pool(name="psB_qa", bufs=3, space="PSUM")
                psB_qb = tc.alloc_tile_pool(name="psB_qb", bufs=3, space="PSUM")
                half = HD // 2
                st = {}       # chunk index -> stage-A state dict
                x_tiles = {}

                def dma_x(i):
                    xtl = bstream.tile([P, NC16, P], BF16, tag="x", bufs=4)
                    nc.sync.dma_start(xtl, xt[i])
                    x_tiles[i] = xtl

                wa_r = wa[:].rearrange("(co ci) j -> ci co j", ci=P)
                wa_tiles = [None] * NC16

                def load_wa(ci_pairs):
                    # pairs: one DMA per two ci tiles (HWDGE slot amortization)
                    for c0 in ci_pairs:
                        wa_pr = wts.tile(
                            [P, 2, JCOLS], BF16, tag=f"wa{c0}", name=f"wa{c0}"
                        )
                        nc.sync.dma_start(wa_pr, wa_r[:, c0 : c0 + 2])
                        wa_tiles[c0] = wa_pr[:, 0, :]
                        wa_tiles[c0 + 1] = wa_pr[:, 1, :]

                def stageB1(j):
                    """DVE rope on raw q/k straight from PSUM; ACT v copy.

                    The RMS scale rs is NOT applied here: rope commutes with a
                    per-row scalar, so rs_q (and 1/sqrt(d)) ride along in the
                    diagonal matrices used by the stageB2 transposes, and rs_k
                    is applied as the per-partition exp scale in phase C.
                    """
                    s = st[j]
                    qa_ps, qb_ps, rs = s["qa"], s["qb"], s["rs"]
                    if "stg" not in s:
                        nc.scalar.copy(v_sb[:, j, :], qb_ps[:, HD : 2 * HD])
                    nc.scalar.copy(rsk_all[:, j : j + 1], rs[:, HG : HG + 1])

                    if "stg" in s:
                        # tail chunks: q/k staged through SBUF right after the
                        # QKV matmuls so the PSUM banks hand off to phase C
                        # without waiting on the DVE rope reads
                        stg = s["stg"]
                        qa_ps = stg[:, 0:QCOLS]
                        qb_ps = stg[:, QCOLS : QCOLS + HD]

                    # k path first: frees the qb PSUM bank early so the
                    # next chunk's qb matmul block isn't blocked on it
                    kswp = bass.AP(
                        tensor=qb_ps.tensor,
                        offset=qb_ps.offset + half,
                        ap=[qb_ps.ap[0], [-half, 2], [1, half]],
                    )
                    t1k = bwork.tile([P, HD], F32, tag="t1k")
                    nc.vector.tensor_tensor(
                        t1k, qb_ps[:, 0:HD], s["csk"], ALU.mult
                    )
                    t2k = bwork.tile([P, HD], F32, tag="t2k")
                    nc.vector.tensor_tensor(
                        t2k.rearrange("p (s e) -> p s e", s=2),
                        kswp,
                        s["snk"].rearrange("p (s e) -> p s e", s=2),
                        ALU.mult,
                    )
                    khat = bwork.tile([P, HD], BF16, tag="khat")
                    nc.vector.tensor_tensor(khat, t1k, t2k, ALU.add)

                    t1q = bwork.tile([P, QCOLS], F32, tag="t1q")
                    t2q = bwork.tile([P, QCOLS], F32, tag="t2q")
                    snq_v = s["snq"].rearrange("p (s e) -> p s e", s=2)
                    for hh in range(HG):
                        h0 = hh * HD
                        # (q * rs_q) * cos  — rs folded into the rope mults
                        nc.vector.scalar_tensor_tensor(
                            t1q[:, h0 : h0 + HD],
                            qa_ps[:, h0 : h0 + HD],
                            rs[:, hh : hh + 1],
                            s["csq"],
                            ALU.mult,
                            ALU.mult,
                        )
                        qswp_h = bass.AP(
                            tensor=qa_ps.tensor,
                            offset=qa_ps.offset + h0 + half,
                            ap=[qa_ps.ap[0], [-half, 2], [1, half]],
                        )
                        nc.vector.scalar_tensor_tensor(
                            t2q[:, h0 : h0 + HD].rearrange(
                                "p (s e) -> p s e", s=2
                            ),
                            qswp_h,
                            rs[:, hh : hh + 1],
                            snq_v,
                            ALU.mult,
                            ALU.mult,
                        )
                    qhat = bwork.tile([P, QCOLS], BF16, tag="qhat")
                    nc.vector.tensor_tensor(qhat, t1q, t2q, ALU.add)
                    s["qhat"], s["khat"] = qhat, khat

                def stageB2(j, tp_pool=None):
                    """PE transposes of qhat/khat + copyback into qT/kT."""
                    s = st.pop(j)
                    t0 = j * P
                    if tp_pool is None:
                        tqk_ps = psB_tp.tile([P, 640], BF16, tag="tp")
                    else:
                        # ride the sc tag's rotation (640 bf16 fits a bank)
                        tqk_ps = tp_pool.tile([P, 640], BF16, tag="sc")
                    for hh in range(HG):
                        nc.tensor.transpose(
                            tqk_ps[:, hh * HD : (hh + 1) * HD],
                            s["qhat"][:, hh * HD : (hh + 1) * HD],
                            ident,
                        )
                    nc.tensor.transpose(tqk_ps[:, QCOLS : QCOLS + HD], s["khat"], ident)
                    nc.scalar.copy(
                        qT[:, :, t0 : t0 + P],
                        tqk_ps[:, 0:QCOLS].rearrange("p (h t) -> p h t", h=HG),
                    )
                    nc.scalar.copy(kT[:, t0 : t0 + P], tqk_ps[:, QCOLS:640])

                def emit_tr(i):
                    # rope tables for chunk i (used in stage B1)
                    s = {}
                    t0 = i * P
                    tb = bstream.tile([P, 4 * HD], F32, tag="tbl", name="tbl_t")
                    nc.sync.dma_start(tb, tbl[t0 : t0 + P, :])
                    s["csq"] = tb[:, 0:HD]
                    s["snq"] = tb[:, HD : 2 * HD]
                    s["csk"] = tb[:, 2 * HD : 3 * HD]
                    s["snk"] = tb[:, 3 * HD : 4 * HD]
                    st[i] = s

                # startup DMA order: wa0 first so the chunk-0 QKV chain can
                # begin ASAP; x0/x1 and the first rope tables interleaved into
                # the wa stream so nothing downstream starves.
                def emit_qkv(i):
                    s = st[i]
                    xT_sb = x_tiles.pop(i)
                    qa_ps = psB_qa.tile([P, QCOLS], F32, tag="qa")
                    qb_ps = psB_qb.tile([P, 2 * HD], F32, tag="qb")
                    for ci in range(NC16):
                        nc.tensor.matmul(
                            qa_ps,
                            xT_sb[:, ci],
                            wa_tiles[ci][:, 0:QCOLS],
                            start=(ci == 0),
                            stop=(ci == NC16 - 1),
                        )
                        nc.tensor.matmul(
                            qb_ps,
                            xT_sb[:, ci],
                            wa_tiles[ci][:, QCOLS:JCOLS],
                            start=(ci == 0),
                            stop=(ci == NC16 - 1),
                        )
                    s["qa"], s["qb"] = qa_ps, qb_ps

                def emit_stg(i):
                    # stage q/k to SBUF + v straight to v_sb, freeing the
                    # qa/qb PSUM banks for the phase-C pools immediately
                    s = st[i]
                    stg = bwork.tile([P, QCOLS + HD], F32, tag="stg", bufs=2)
                    nc.scalar.copy(stg[:, 0:QCOLS], s["qa"])
                    nc.scalar.copy(stg[:, QCOLS : QCOLS + HD], s["qb"][:, 0:HD])
                    nc.scalar.copy(v_sb[:, i, :], s["qb"][:, HD : 2 * HD])
                    s["stg"] = stg

                def emit_stats(i):
                    s = st[i]
                    qa_v = s["qa"]
                    qk_v = s["qb"][:, 0:HD]
                    ss = bwork.tile([P, HG + 1], F32, tag="ss")
                    for hh in range(HG + 1):
                        src = (
                            qa_v[:, hh * HD : (hh + 1) * HD] if hh < HG else qk_v
                        )
                        sq = bwork.tile([P, HD], F32, tag="sq")
                        nc.scalar.activation(
                            sq, src, AF.Square, accum_out=ss[:, hh : hh + 1]
                        )
                    rt = bwork.tile([P, HG + 1], F32, tag="rt")
                    nc.scalar.activation(
                        rt, ss, AF.Sqrt, bias=eps_sb, scale=1.0 / HD
                    )
                    rs = bwork.tile([P, HG + 1], F32, tag="rs")
                    nc.vector.reciprocal(rs, rt)
                    s["rs"] = rs

                # startup: interleave wa pairs with x0 quarters so the
                # chunk-0 ci-accumulation paces smoothly off the DMA stream
                xtl0 = bstream.tile([P, NC16, P], BF16, tag="x", bufs=4)
                x_tiles[0] = xtl0

                def x0_part(q4):
                    nc.sync.dma_start(
                        xtl0[:, 4 * q4 : 4 * q4 + 4, :],
                        xt[0, :, 4 * q4 : 4 * q4 + 4, :],
                    )

                load_wa([0])
                x0_part(0)
                load_wa([2])
                x0_part(1)
                emit_tr(0)
                load_wa([4])
                x0_part(2)
                x0_part(3)
                load_wa([6])
                dma_x(1)
                emit_tr(1)
                load_wa([8, 10, 12, 14])
                dma_x(2)
                emit_tr(2)
                dma_x(3)
                nc.sync.dma_start(dm1_sb, dm1[:])

                for i in range(NT):
                    if i >= 2 and i + 2 < NT:
                        dma_x(i + 2)
                    if i in (8, 10):
                        # prefetch wp in halves so phase C isn't gated on it
                        hh0 = (i - 8) // 2 * 2
                        nc.sync.dma_start(
                            wp_sb[:, hh0 : hh0 + 2, :],
                            wp[:]
                            .rearrange("(h d) e -> d h e", d=P)[:, hh0 : hh0 + 2],
                        )

                    # rope for chunk i-1 (ACT/DVE overlap the MMs)
                    if i >= 1:
                        stageB1(i - 1)
                    emit_qkv(i)
                    if i == NT - 1:
                        emit_stg(i)
                    if i >= 2 and i + 1 < NT:
                        emit_tr(i + 1)
                    # chunk i-2 q/k transposes (lag 2: its rope is certainly
                    # done, so the PE never waits on the DVE chain)
                    if i >= 2:
                        stageB2(i - 2)
                    emit_stats(i)

            # ---------------- Phase C+D interleaved ----------------------
            # bf16 matmuls have no small-width penalty: all stages run on the
            # exact 128-granular staircase region. Opened while phase B's
            # last-chunk tail is still pending: Ti=0/1 only read chunks 0-7,
            # so their matmuls overlap the chunk-15 rope/transpose drain.
            OFFS = [0, 128, 256, 384]
            if True:
                cwork = tc.alloc_tile_pool(name="cwork", bufs=6, side="right")
                dout = tc.alloc_tile_pool(name="dout", bufs=4, side="right")

                # projection emitted as "filler" e-blocks interleaved into
                # the attention S-stream: the S-stream is ACT(exp)-paced, so
                # proj matmuls soak up the PE slack instead of running as
                # dedicated PE-bound blocks that starve ACT.
                proj_fill = []

                def queue_proj(Tb, part, split=False):
                    tci = 4 * Tb + part
                    t0 = tci * P
                    state = {}

                    def mk(e):
                        def f():
                            if "o" not in state:
                                state["o"] = dout.tile(
                                    [P, C], BF16, tag="o_sb", name=f"o_{tci}"
                                )
                            o_sb = state["o"]
                            o_ps = psC_sc.tile(
                                [P, 512], F32, tag="sc", name=f"op_{tci}_{e}"
                            )
                            for hh2 in range(HG):
                                nc.tensor.matmul(
                                    o_ps,
                                    yT[:, hh2, t0 : t0 + P],
                                    wp_sb[:, hh2, e * 512 : (e + 1) * 512],
                                    start=(hh2 == 0),
                                    stop=(hh2 == HG - 1),
                                )
                            nc.vector.tensor_copy(
                                o_sb[:, e * 512 : (e + 1) * 512], o_ps
                            )
                            if split:
                                nc.sync.dma_start(
                                    out[t0 : t0 + P, e * 512 : (e + 1) * 512],
                                    o_sb[:, e * 512 : (e + 1) * 512],
                                )
                            elif e == 3:
                                nc.sync.dma_start(out[t0 : t0 + P, :], o_sb)

                        return f

                    for e in range(4):
                        proj_fill.append(mk(e))

                LAG = 3  # sc/exp run ahead of pv/den by this many chunks

                def make_head(Ti, h, scp, accp):
                    """Returns (warmup, rest): warmup emits the first LAG
                    score/exp groups; rest emits the pv/den stream (injecting
                    the NEXT head's warmup before the final pv/den pair) and
                    the normalize."""
                    tt0 = Ti * 512
                    nS = 4 * Ti + 4
                    exs = {}
                    exqs = {}

                    def emit_sc(S):
                        r = S - 4 * Ti
                        off = OFFS[r] if r >= 0 else 0
                        sc_ps = scp.tile([P, 512], F32, tag="sc")
                        nc.tensor.matmul(
                            sc_ps[:, off:512],
                            kT[:, S * P : (S + 1) * P],
                            qT[:, h, tt0 + off : tt0 + 512],
                            start=True,
                            stop=True,
                        )
                        ex = cwork.tile([P, 512], BF16, tag="ex")
                        nc.scalar.activation(
                            ex[:, off:512], sc_ps[:, off:512], AF.Exp,
                            scale=rsk_all[:, S : S + 1],
                        )
                        if r >= 0:
                            nc.vector.tensor_tensor(
                                ex[:, off : off + P],
                                ex[:, off : off + P],
                                dm1_sb,
                                ALU.mult,
                            )
                        exs[S] = (ex, off)
                        # quad pre-sum for the denominator (full-width tiles
                        # only): runs LAG chunks ahead of the den matmul so
                        # the DVE adds never stall the PE
                        if S < 4 * Ti:
                            qi, qpos = divmod(S, 4)
                            if qpos == 0:
                                exqs[qi] = ex
                            elif qpos == 1:
                                exq = cwork.tile(
                                    [P, 512], BF16, tag="exq", bufs=2
                                )
                                nc.vector.tensor_tensor(
                                    exq, exqs[qi], ex, ALU.add
                                )
                                exqs[qi] = exq
                            else:
                                nc.vector.tensor_tensor(
                                    exqs[qi], exqs[qi], ex, ALU.add
                                )

                    def warmup():
                        for S in range(min(LAG, nS)):
                            emit_sc(S)

                    def rest(next_warmup):
                        yt_ps = accp.tile([P, 512], F32, tag="yt")
                        den_ps = accp.tile([P, 512], F32, tag="den")
                        # denominator: the 4*Ti full-width exp tiles are
                        # pre-summed in quads on DVE (bf16 2x mode) so the PE
                        # streams each quad once instead of four times; the 4
                        # diagonal (staircase) tiles go straight to PE.
                        n_den = Ti + 4
                        den_i = 0

                        def den_mm(src, off_):
                            nonlocal den_i
                            nc.tensor.matmul(
                                den_ps[:, off_:512],
                                ones,
                                src[:, off_:512],
                                start=(den_i == 0),
                                stop=(den_i == n_den - 1),
                            )
                            den_i += 1

                        for S in range(nS):
                            if S + LAG < nS:
                                emit_sc(S + LAG)
                            if S == nS - 1 and next_warmup is not None:
                                next_warmup()
                            ex, off = exs.pop(S)
                            nc.tensor.matmul(
                                yt_ps[:, off:512],
                                v_sb[:, S, :],
                                ex[:, off:512],
                                start=(S == 0),
                                stop=(S == nS - 1),
                            )
                            if S < 4 * Ti:
                                if S % 4 == 3:
                                    den_mm(exqs.pop(S // 4), 0)
                            else:
                                den_mm(ex, off)
                            if proj_fill and S % 2 == 1:
                                proj_fill.pop(0)()
                        denr = cwork.tile([P, 512], F32, tag="denr")
                        scr = cwork.tile([P, 512], F32, tag="scr")
                        nc.vector.reciprocal_approx_accurate(denr, den_ps, scr)
                        nc.vector.tensor_tensor(
                            yT[:, h, tt0 : tt0 + 512], yt_ps, denr, ALU.mult
                        )

                    return warmup, rest

                # phase-B tail: chunk-15's rope reads only the SBUF staging
                # copy, so every psB PSUM bank is free right after the stg
                # copies — release before the tail so phase C's pools can
                # claim all 8 banks immediately. The tail rope + transposes
                # are emitted AFTER Ti=0 so the PE stream isn't serialized
                # behind the DVE chain.
                psB_qb.release()
                psB_qa.release()
                psB_tp.release()

                psC_sc = tc.alloc_tile_pool(
                    name="psC_sc", bufs=4, space="PSUM", side="right"
                )
                psC_acc = tc.alloc_tile_pool(name="psC_acc", bufs=2, space="PSUM")

                # software-pipelined head stream: each head's final pv/den
                # pair is preceded by the next head's warmup scores, so the
                # exp latency never surfaces at head boundaries.
                prev_rest = None

                def run_head(Ti, h):
                    nonlocal prev_rest
                    w, r = make_head(Ti, h, psC_sc, psC_acc)
                    if prev_rest is None:
                        w()
                    else:
                        prev_rest(w)
                    prev_rest = r

                for h in range(HG):
                    run_head(0, h)
                stageB1(NT - 1)
                stageB2(NT - 2, tp_pool=psC_sc)
                stageB2(NT - 1, tp_pool=psC_sc)
                bwork.release()
                bstream.release()
                wts.release()
                for Ti in (1, 2, 3):
                    for h in range(HG):
                        if h == 1:
                            for part in range(4):
                                queue_proj(Ti - 1, part)
                        run_head(Ti, h)
                prev_rest(None)
                for part in range(4):
                    queue_proj(3, part, split=(part >= 2))
                while proj_fill:
                    proj_fill.pop(0)()
                psC_acc.release()
                psC_sc.release()
                dout.release()
                cwork.release()



    nc.finalize()
    return nc


def _host_tables(q_norm_w, k_norm_w):
    """RoPE cos/sin tables in [t, d] layout with norm weights folded in."""
    half = HD // 2
    inv_freq = (
        1.0 / (ROPE_BASE ** (np.arange(0, half, dtype=np.float32) / half))
    ).astype(np.float32)
    ang = np.arange(T, dtype=np.float32)[:, None] * inv_freq[None, :]  # [T, half]
    cos = np.cos(ang).astype(np.float32)
    sin = np.sin(ang).astype(np.float32)
    cos2 = np.concatenate([cos, cos], axis=1)           # [T, 128]
    sin2 = np.concatenate([-sin, sin], axis=1)          # [T, 128]
    # q tables carry the softmax 1/sqrt(d) scale (rs_q rides in the rope
    # mults, rs_k in the exp scale)
    csq1 = cos2 * q_norm_w[None, :] * SCALE
    snq1 = sin2 * q_norm_w[None, :] * SCALE
    csq = np.ascontiguousarray(csq1, dtype=np.float32)  # [T, 128]
    snq = np.ascontiguousarray(snq1, dtype=np.float32)
    csk = (cos2 * k_norm_w[None, :]).astype(np.float32)
    snk = (sin2 * k_norm_w[None, :]).astype(np.float32)
    return np.ascontiguousarray(
        np.concatenate([csq, snq, csk, snk], axis=1)
    )


def _host_masks():
    idx = np.arange(P)
    stair = (idx[None, :] // BLOCK >= idx[:, None] // BLOCK).astype(NPBF16)
    return np.ascontiguousarray(stair)


def _host_x(xb):
    """[T, C] -> [NT, 128c, NC16, 128t] blocked-transposed bf16 layout."""
    return np.ascontiguousarray(
        xb.reshape(NT, P, NC16, P).transpose(0, 3, 2, 1).astype(NPBF16)
    )


_nc_cache = None


def kernel(x, W_attn, W_proj, q_norm_w, k_norm_w):
    global _nc_cache
    x = np.asarray(x, dtype=np.float32)
    W_attn = np.asarray(W_attn, dtype=np.float32)
    W_proj = np.asarray(W_proj, dtype=np.float32)
    q_norm_w = np.asarray(q_norm_w, dtype=np.float32)
    k_norm_w = np.asarray(k_norm_w, dtype=np.float32)
    B = x.shape[0]

    tblarr = _host_tables(q_norm_w, k_norm_w)
    dm1 = _host_masks()
    xts = [_host_x(x[b]) for b in range(B)]

    in_maps = []
    for core in range(8):
        b, g = divmod(core, 4)
        wa_core = np.concatenate(
            [
                W_attn[:, g * QCOLS : (g + 1) * QCOLS],
                W_attn[:, C + g * HD : C + (g + 1) * HD],
                W_attn[:, C + N_KV * HD + g * HD : C + N_KV * HD + (g + 1) * HD],
            ],
            axis=1,
        )
        wp_core = W_proj[g * QCOLS : (g + 1) * QCOLS, :]
        in_maps.append(
            {
                "xt": xts[b],
                "wa": np.ascontiguousarray(wa_core.astype(NPBF16)),
                "wp": np.ascontiguousarray(wp_core.astype(NPBF16)),
                "tbl": tblarr,
                "dm1": dm1,
            }
        )

    if _nc_cache is None:
        _nc_cache = build_nc()
    res = run_bass_kernel_spmd(_nc_cache, in_maps, core_ids=list(range(8)))

    out = np.zeros((B, T, C), dtype=np.float32)
    for core in range(8):
        b = core // 4
        out[b] += res.results[core]["out"].astype(np.float32)
    return out



# revision 144
# speedup vs baseline: 1.9064x; 1.9064x over previous
"""Block-causal GQA attention layer on 8 Trainium2 NeuronCores.

Sharding: 8 cores = batch(2) x head-group(4). Core c handles batch b=c//4 and
head group g=c%4 (q heads 4g..4g+3, kv head g). W_attn is column-sharded by
head group, W_proj row-sharded; each core computes a partial [T, C] output
(bf16, upcast on host) and the host sums the 4 partials per batch element.

All matmuls run in bf16 (fp32 PSUM accumulate); rel err vs the fp32
reference is ~6e-3. x is pre-transposed on the host into a per-t-chunk
blocked layout so no PE transposes of x are needed, and the rope cos/sin
tables (with qk-norm weights and the 1/sqrt(d) softmax scale folded in) are
host-precomputed.

Per-core device pipeline:
  B) software-pipelined over 16 t-chunks: QKV matmuls (x-chunk stationary,
     W_attn streaming, fp32 PSUM) -> RMS stats (ACT square-accum) ->
     RoPE applied by DVE scalar_tensor_tensor ops reading raw q/k straight
     from PSUM, with the per-row rsqrt folded into the rope multiplies
     (rope commutes with a per-row scalar); rs_k instead rides as the
     per-partition exp scale in phase C. PE transposes (lag 2) produce
     qT/kT in [d, t] layout. The last chunk is staged through SBUF so all
     PSUM banks hand off to phase C without waiting on its rope, and its
     transposes are emitted after Ti=0.
  C) per 512-wide T-block Ti and head h: scores sT = kT.T @ qT on the exact
     128-granular block-causal staircase, exp on ACT (per-partition rs_k
     scale) into bf16, staircase mask multiply (DVE, bf16 2x) on diagonal
     tiles only, PV matmuls, and a denominator ones-matmul where groups of
     four full-width exp tiles are pre-summed on DVE so the PE streams each
     quad once. sc/exp run LAG=3 chunks ahead of PV; each head's final
     pv/den pair is preceded by the next head's warmup so exp latency never
     surfaces. approx-reciprocal normalize writes yT (bf16).
  D) output projection emitted as filler e-blocks interleaved into the
     (ACT-paced) attention S-stream, one per two S-chunks, with batched
     per-t-chunk output DMAs (bf16).
"""

import numpy as np

import concourse.bacc as bacc
import concourse.bass as bass
import concourse.tile as tile
import concourse.mybir as mybir
from concourse.bass_utils import run_bass_kernel_spmd
from concourse.masks import make_identity

P = 128
T = 2048
C = 2048
N_HEAD = 16
N_KV = 4
HD = 128          # head dim
HG = N_HEAD // N_KV  # heads per group = 4
BLOCK = 16
EPS = 1e-5
ROPE_BASE = 500000.0
QCOLS = HG * HD   # 512 q cols per core
JCOLS = QCOLS + 2 * HD  # 768 qkv cols per core
NT = T // P       # 16 t-chunks
NC16 = C // P     # 16 c-chunks
SCALE = 1.0 / float(np.sqrt(np.float32(HD)))

F32 = mybir.dt.float32
BF16 = mybir.dt.bfloat16
NPBF16 = mybir.dt.np(mybir.dt.bfloat16)
AF = mybir.ActivationFunctionType
ALU = mybir.AluOpType


def build_nc():
    nc = bacc.Bacc("TRN2", target_bir_lowering=False)

    # xt[i, c, ci, t] = x[i*128 + t, ci*128 + c]: per-t-chunk pre-transposed
    # blocked layout prepared on host; one contiguous [128, 2048] DMA per chunk.
    xt = nc.dram_tensor("xt", [NT, P, NC16, P], BF16, kind="ExternalInput")
    wa = nc.dram_tensor("wa", [C, JCOLS], BF16, kind="ExternalInput")
    wp = nc.dram_tensor("wp", [QCOLS, C], BF16, kind="ExternalInput")
    # rope tables packed [csq | snq | csk | snk] along the free axis
    tbl = nc.dram_tensor("tbl", [T, 4 * HD], F32, kind="ExternalInput")
    dm1 = nc.dram_tensor("dm1", [P, P], BF16, kind="ExternalInput")
    out = nc.dram_tensor("out", [T, C], BF16, kind="ExternalOutput")

    with tile.TileContext(nc) as tc:
        with tc.tile_pool(name="persist", bufs=1) as persist:
            ident_f = persist.tile([P, P], F32)
            make_identity(nc, ident_f)
            ident = persist.tile([P, P], BF16)
            nc.vector.tensor_copy(ident, ident_f)
            rsk_all = persist.tile([P, NT], F32)
            ones = persist.tile([P, P], BF16)
            nc.vector.memset(ones, 1.0)
            dm1_sb = persist.tile([P, P], BF16)
            eps_sb = persist.tile([P, 1], F32)
            nc.vector.memset(eps_sb, EPS)

            qT = persist.tile([P, HG, T], BF16)     # [d, h, t]
            kT = persist.tile([P, T], BF16)         # [d, t]
            v_sb = persist.tile([P, NT, HD], BF16)  # [s_in_chunk, s_chunk, d']
            yT = persist.tile([P, HG, T], BF16)     # [d', h, t]
            wp_sb = persist.tile([P, HG, C], BF16)

            # ---------------- Phase B (software-pipelined) ---------------
            # Explicit pool lifetimes (released in stack order per side):
            # phase B PSUM = tp2 + qa3 + qb3 = 8 banks; released before the
            # staged chunk-15 tail so phase C's pools (sc4 + yt2/den2) can
            # claim all 8 banks with no handoff stall.
            if True:
                wts = tc.alloc_tile_pool(name="wts", bufs=1)
                bstream = tc.alloc_tile_pool(name="bstream", bufs=3)
                bwork = tc.alloc_tile_pool(name="bwork", bufs=4)
                psB_tp = tc.alloc_tile_pool(name="psB_tp", bufs=2, space="PSUM")
                psB_qa = tc.alloc_tile_pool(name="psB_qa", bufs=3, space="PSUM")
                psB_qb = tc.alloc_tile_pool(name="psB_qb", bufs=3, space="PSUM")
                half = HD // 2
                st = {}       # chunk index -> stage-A state dict
                x_tiles = {}

                def dma_x(i):
                    xtl = bstream.tile([P, NC16, P], BF16, tag="x", bufs=4)
                    nc.sync.dma_start(xtl, xt[i])
                    x_tiles[i] = xtl

                wa_r = wa[:].rearrange("(co ci) j -> ci co j", ci=P)
                wa_tiles = [None] * NC16

                def load_wa(ci_pairs):
                    # pairs: one DMA per two ci tiles (HWDGE slot amortization)
                    for c0 in ci_pairs:
                        wa_pr = wts.tile(
                            [P, 2, JCOLS], BF16, tag=f"wa{c0}", name=f"wa{c0}"
                        )
                        nc.sync.dma_start(wa_pr, wa_r[:, c0 : c0 + 2])
                        wa_tiles[c0] = wa_pr[:, 0, :]
                        wa_tiles[c0 + 1] = wa_pr[:, 1, :]

                def stageB1(j):
                    """DVE rope on raw q/k straight from PSUM; ACT v copy.

                    The RMS scale rs is NOT applied here: rope commutes with a
                    per-row scalar, so rs_q (and 1/sqrt(d)) ride along in the
                    diagonal matrices used by the stageB2 transposes, and rs_k
                    is applied as the per-partition exp scale in phase C.
                    """
                    s = st[j]
                    qa_ps, qb_ps, rs = s["qa"], s["qb"], s["rs"]
                    if "stg" not in s:
                        nc.scalar.copy(v_sb[:, j, :], qb_ps[:, HD : 2 * HD])
                    nc.scalar.copy(rsk_all[:, j : j + 1], rs[:, HG : HG + 1])

                    if "stg" in s:
                        # tail chunks: q/k staged through SBUF right after the
                        # QKV matmuls so the PSUM banks hand off to phase C
                        # without waiting on the DVE rope reads
                        stg = s["stg"]
                        qa_ps = stg[:, 0:QCOLS]
                        qb_ps = stg[:, QCOLS : QCOLS + HD]

                    # k path first: frees the qb PSUM bank early so the
                    # next chunk's qb matmul block isn't blocked on it
                    kswp = bass.AP(
                        tensor=qb_ps.tensor,
                        offset=qb_ps.offset + half,
                        ap=[qb_ps.ap[0], [-half, 2], [1, half]],
                    )
                    t1k = bwork.tile([P, HD], F32, tag="t1k")
                    nc.vector.tensor_tensor(
                        t1k, qb_ps[:, 0:HD], s["csk"], ALU.mult
                    )
                    t2k = bwork.tile([P, HD], F32, tag="t2k")
                    nc.vector.tensor_tensor(
                        t2k.rearrange("p (s e) -> p s e", s=2),
                        kswp,
                        s["snk"].rearrange("p (s e) -> p s e", s=2),
                        ALU.mult,
                    )
                    khat = bwork.tile([P, HD], BF16, tag="khat")
                    nc.vector.tensor_tensor(khat, t1k, t2k, ALU.add)

                    t1q = bwork.tile([P, QCOLS], F32, tag="t1q")
                    t2q = bwork.tile([P, QCOLS], F32, tag="t2q")
                    snq_v = s["snq"].rearrange("p (s e) -> p s e", s=2)
                    for hh in range(HG):
                        h0 = hh * HD
                        # (q * rs_q) * cos  — rs folded into the rope mults
                        nc.vector.scalar_tensor_tensor(
                            t1q[:, h0 : h0 + HD],
                            qa_ps[:, h0 : h0 + HD],
                            rs[:, hh : hh + 1],
                            s["csq"],
                            ALU.mult,
                            ALU.mult,
                        )
                        qswp_h = bass.AP(
                            tensor=qa_ps.tensor,
                            offset=qa_ps.offset + h0 + half,
                            ap=[qa_ps.ap[0], [-half, 2], [1, half]],
                        )
                        nc.vector.scalar_tensor_tensor(
                            t2q[:, h0 : h0 + HD].rearrange(
                                "p (s e) -> p s e", s=2
                            ),
                            qswp_h,
                            rs[:, hh : hh + 1],
                            snq_v,
                            ALU.mult,
                            ALU.mult,
                        )
                    qhat = bwork.tile([P, QCOLS], BF16, tag="qhat")
                    nc.vector.tensor_tensor(qhat, t1q, t2q, ALU.add)
                    s["qhat"], s["khat"] = qhat, khat

                def stageB2(j, tp_pool=None):
                    """PE transposes of qhat/khat + copyback into qT/kT."""
                    s = st.pop(j)
                    t0 = j * P
                    if tp_pool is None:
                        tqk_ps = psB_tp.tile([P, 640], BF16, tag="tp")
                    else:
                        # ride the sc tag's rotation (640 bf16 fits a bank)
                        tqk_ps = tp_pool.tile([P, 640], BF16, tag="sc")
                    for hh in range(HG):
                        nc.tensor.transpose(
                            tqk_ps[:, hh * HD : (hh + 1) * HD],
                            s["qhat"][:, hh * HD : (hh + 1) * HD],
                            ident,
                        )
                    nc.tensor.transpose(tqk_ps[:, QCOLS : QCOLS + HD], s["khat"], ident)
                    nc.scalar.copy(
                        qT[:, :, t0 : t0 + P],
                        tqk_ps[:, 0:QCOLS].rearrange("p (h t) -> p h t", h=HG),
                    )
                    nc.scalar.copy(kT[:, t0 : t0 + P], tqk_ps[:, QCOLS:640])

                def emit_tr(i):
                    # rope tables for chunk i (used in stage B1)
                    s = {}
                    t0 = i * P
                    tb = bstream.tile([P, 4 * HD], F32, tag="tbl", name="tbl_t")
                    nc.sync.dma_start(tb, tbl[t0 : t0 + P, :])
                    s["csq"] = tb[:, 0:HD]
                    s["snq"] = tb[:, HD : 2 * HD]
                    s["csk"] = tb[:, 2 * HD : 3 * HD]
                    s["snk"] = tb[:, 3 * HD : 4 * HD]
                    st[i] = s

                # startup DMA order: wa0 first so the chunk-0 QKV chain can
                # begin ASAP; x0/x1 and the first rope tables interleaved into
                # the wa stream so nothing downstream starves.
                def emit_qkv(i):
                    s = st[i]
                    xT_sb = x_tiles.pop(i)
                    qa_ps = psB_qa.tile([P, QCOLS], F32, tag="qa")
                    qb_ps = psB_qb.tile([P, 2 * HD], F32, tag="qb")
                    for ci in range(NC16):
                        nc.tensor.matmul(
                            qa_ps,
                            xT_sb[:, ci],
                            wa_tiles[ci][:, 0:QCOLS],
                            start=(ci == 0),
                            stop=(ci == NC16 - 1),
                        )
                        nc.tensor.matmul(
                            qb_ps,
                            xT_sb[:, ci],
                            wa_tiles[ci][:, QCOLS:JCOLS],
                            start=(ci == 0),
                            stop=(ci == NC16 - 1),
                        )
                    s["qa"], s["qb"] = qa_ps, qb_ps

                def emit_stg(i):
                    # stage q/k to SBUF + v straight to v_sb, freeing the
                    # qa/qb PSUM banks for the phase-C pools immediately
                    s = st[i]
                    stg = bwork.tile([P, QCOLS + HD], F32, tag="stg", bufs=2)
                    nc.scalar.copy(stg[:, 0:QCOLS], s["qa"])
                    nc.scalar.copy(stg[:, QCOLS : QCOLS + HD], s["qb"][:, 0:HD])
                    nc.scalar.copy(v_sb[:, i, :], s["qb"][:, HD : 2 * HD])
                    s["stg"] = stg

                def emit_stats(i):
                    s = st[i]
                    qa_v = s["qa"]
                    qk_v = s["qb"][:, 0:HD]
                    ss = bwork.tile([P, HG + 1], F32, tag="ss")
                    for hh in range(HG + 1):
                        src = (
                            qa_v[:, hh * HD : (hh + 1) * HD] if hh < HG else qk_v
                        )
                        sq = bwork.tile([P, HD], F32, tag="sq")
                        nc.scalar.activation(
                            sq, src, AF.Square, accum_out=ss[:, hh : hh + 1]
                        )
                    rt = bwork.tile([P, HG + 1], F32, tag="rt")
                    nc.scalar.activation(
                        rt, ss, AF.Sqrt, bias=eps_sb, scale=1.0 / HD
                    )
                    rs = bwork.tile([P, HG + 1], F32, tag="rs")
                    nc.vector.reciprocal(rs, rt)
                    s["rs"] = rs

                # startup: interleave wa pairs with x0 quarters so the
                # chunk-0 ci-accumulation paces smoothly off the DMA stream
                xtl0 = bstream.tile([P, NC16, P], BF16, tag="x", bufs=4)
                x_tiles[0] = xtl0

                def x0_part(q4):
                    nc.sync.dma_start(
                        xtl0[:, 4 * q4 : 4 * q4 + 4, :],
                        xt[0, :, 4 * q4 : 4 * q4 + 4, :],
                    )

                load_wa([0])
                x0_part(0)
                load_wa([2])
                x0_part(1)
                emit_tr(0)
                load_wa([4])
                x0_part(2)
                x0_part(3)
                load_wa([6])
                dma_x(1)
                emit_tr(1)
                load_wa([8, 10, 12, 14])
                dma_x(2)
                emit_tr(2)
                dma_x(3)
                nc.sync.dma_start(dm1_sb, dm1[:])

                for i in range(NT):
                    if i >= 2 and i + 2 < NT:
                        dma_x(i + 2)
                    if i in (8, 10):
                        # prefetch wp in halves so phase C isn't gated on it
                        hh0 = (i - 8) // 2 * 2
                        nc.sync.dma_start(
                            wp_sb[:, hh0 : hh0 + 2, :],
                            wp[:]
                            .rearrange("(h d) e -> d h e", d=P)[:, hh0 : hh0 + 2],
                        )

                    # rope for chunk i-1 (ACT/DVE overlap the MMs)
                    if i >= 1:
                        stageB1(i - 1)
                    emit_qkv(i)
                    if i == NT - 1:
                        emit_stg(i)
                    if i >= 2 and i + 1 < NT:
                        emit_tr(i + 1)
                    # chunk i-2 q/k transposes (lag 2: its rope is certainly
                    # done, so the PE never waits on the DVE chain)
                    if i >= 2:
                        stageB2(i - 2)
                    emit_stats(i)

            # ---------------- Phase C+D interleaved ----------------------
            # bf16 matmuls have no small-width penalty: all stages run on the
            # exact 128-granular staircase region. Opened while phase B's
            # last-chunk tail is still pending: Ti=0/1 only read chunks 0-7,
            # so their matmuls overlap the chunk-15 rope/transpose drain.
            OFFS = [0, 128, 256, 384]
            if True:
                cwork = tc.alloc_tile_pool(name="cwork", bufs=6, side="right")
                dout = tc.alloc_tile_pool(name="dout", bufs=4, side="right")

                # projection emitted as "filler" e-blocks interleaved into
                # the attention S-stream: the S-stream is ACT(exp)-paced, so
                # proj matmuls soak up the PE slack instead of running as
                # dedicated PE-bound blocks that starve ACT.
                proj_fill = []

                def queue_proj(Tb, part, split=False):
                    tci = 4 * Tb + part
                    t0 = tci * P
                    state = {}

                    def mk(e):
                        def f():
                            if "o" not in state:
                                state["o"] = dout.tile(
                                    [P, C], BF16, tag="o_sb", name=f"o_{tci}"
                                )
                            o_sb = state["o"]
                            o_ps = psC_sc.tile(
                                [P, 512], F32, tag="sc", name=f"op_{tci}_{e}"
                            )
                            for hh2 in range(HG):
                                nc.tensor.matmul(
                                    o_ps,
                                    yT[:, hh2, t0 : t0 + P],
                                    wp_sb[:, hh2, e * 512 : (e + 1) * 512],
                                    start=(hh2 == 0),
                                    stop=(hh2 == HG - 1),
                                )
                            nc.vector.tensor_copy(
                                o_sb[:, e * 512 : (e + 1) * 512], o_ps
                            )
                            if split:
                                nc.sync.dma_start(
                                    out[t0 : t0 + P, e * 512 : (e + 1) * 512],
                                    o_sb[:, e * 512 : (e + 1) * 512],
                                )
                            elif e == 3:
                                nc.sync.dma_start(out[t0 : t0 + P, :], o_sb)

                        return f

                    for e in range(4):
                        proj_fill.append(mk(e))

                LAG = 3  # sc/exp run ahead of pv/den by this many chunks

                def make_head(Ti, h, scp, accp):
                    """Returns (warmup, rest): warmup emits the first LAG
                    score/exp groups; rest emits the pv/den stream (injecting
                    the NEXT head's warmup before the final pv/den pair) and
                    the normalize."""
                    tt0 = Ti * 512
                    nS = 4 * Ti + 4
                    exs = {}
                    exqs = {}

                    def emit_sc(S):
                        r = S - 4 * Ti
                        off = OFFS[r] if r >= 0 else 0
                        sc_ps = scp.tile([P, 512], F32, tag="sc")
                        nc.tensor.matmul(
                            sc_ps[:, off:512],
                            kT[:, S * P : (S + 1) * P],
                            qT[:, h, tt0 + off : tt0 + 512],
                            start=True,
                            stop=True,
                        )
                        ex = cwork.tile([P, 512], BF16, tag="ex")
                        nc.scalar.activation(
                            ex[:, off:512], sc_ps[:, off:512], AF.Exp,
                            scale=rsk_all[:, S : S + 1],
                        )
                        if r >= 0:
                            nc.vector.tensor_tensor(
                                ex[:, off : off + P],
                                ex[:, off : off + P],
                                dm1_sb,
                                ALU.mult,
                            )
                        exs[S] = (ex, off)
                        # denominator pre-sum on DVE (bf16 2x), LAG chunks
                        # ahead of the den matmul so the adds never stall the
                        # PE. Full-width tiles sum in quads; the 4 diagonal
                        # (staircase) tiles fold their valid [off:512]
                        # regions into the last quad (or a fresh copy for
                        # Ti=0), so the PE streams each 512-col quad once and
                        # the diagonals cost no extra den matmuls at all.
                        if Ti == 0:
                            if S == 0:
                                exq = cwork.tile(
                                    [P, 512], BF16, tag="exq", bufs=2
                                )
                                nc.vector.tensor_copy(exq, ex)
                                exqs[0] = exq
                            elif r < 3:
                                nc.vector.tensor_tensor(
                                    exqs[0][:, off:512],
                                    exqs[0][:, off:512],
                                    ex[:, off:512],
                                    ALU.add,
                                )
                        elif r < 0:
                            qi, qpos = divmod(S, 4)
                            if qpos == 0:
                                exqs[qi] = ex
                            elif qpos == 1:
                                exq = cwork.tile(
                                    [P, 512], BF16, tag="exq", bufs=2
                                )
                                nc.vector.tensor_tensor(
                                    exq, exqs[qi], ex, ALU.add
                                )
                                exqs[qi] = exq
                            else:
                                nc.vector.tensor_tensor(
                                    exqs[qi], exqs[qi], ex, ALU.add
                                )
                        elif r < 3:
                            nc.vector.tensor_tensor(
                                exqs[Ti - 1][:, off:512],
                                exqs[Ti - 1][:, off:512],
                                ex[:, off:512],
                                ALU.add,
                            )
                        # r == 3 goes straight to PE as a 128-col den matmul
                        # so the folded quad's den can close one S earlier

                    def warmup():
                        for S in range(min(LAG, nS)):
                            emit_sc(S)

                    def rest(next_warmup):
                        yt_ps = accp.tile([P, 512], F32, tag="yt")
                        den_ps = accp.tile([P, 512], F32, tag="den")
                        # den matmuls: one per quad (quads 0..Ti-2 at their
                        # boundary, the last quad — which also carries the
                        # diagonal tiles — at the end of the S loop)
                        n_den = max(Ti, 1) + 1
                        den_i = 0

                        def den_mm(src, off_=0):
                            nonlocal den_i
                            nc.tensor.matmul(
                                den_ps[:, off_:512],
                                ones,
                                src[:, off_:512],
                                start=(den_i == 0),
                                stop=(den_i == n_den - 1),
                            )
                            den_i += 1

                        for S in range(nS):
                            if S + LAG < nS:
                                emit_sc(S + LAG)
                            if S == nS - 1 and next_warmup is not None:
                                next_warmup()
                            ex, off = exs.pop(S)
                            nc.tensor.matmul(
                                yt_ps[:, off:512],
                                v_sb[:, S, :],
                                ex[:, off:512],
                                start=(S == 0),
                                stop=(S == nS - 1),
                            )
                            if S < 4 * (Ti - 1) and S % 4 == 3:
                                den_mm(exqs.pop(S // 4))
                            if S == nS - 2:
                                den_mm(exqs.pop(max(Ti - 1, 0)))
                            if S == nS - 1:
                                den_mm(ex, OFFS[3])
                            if proj_fill and S % 2 == 1:
                                proj_fill.pop(0)()
                        denr = cwork.tile([P, 512], F32, tag="denr")
                        # ~18 correct bits: far inside the bf16 error budget
                        nc.vector.reciprocal_approx_fast(denr, den_ps)
                        nc.vector.tensor_tensor(
                            yT[:, h, tt0 : tt0 + 512], yt_ps, denr, ALU.mult
                        )

                    return warmup, rest

                # phase-B tail: chunk-15's rope reads only the SBUF staging
                # copy, so every psB PSUM bank is free right after the stg
                # copies — release before the tail so phase C's pools can
                # claim all 8 banks immediately. The tail rope + transposes
                # are emitted AFTER Ti=0 so the PE stream isn't serialized
                # behind the DVE chain.
                psB_qb.release()
                psB_qa.release()
                psB_tp.release()

                psC_sc = tc.alloc_tile_pool(
                    name="psC_sc", bufs=4, space="PSUM", side="right"
                )
                psC_acc = tc.alloc_tile_pool(name="psC_acc", bufs=2, space="PSUM")

                # software-pipelined head stream: each head's final pv/den
                # pair is preceded by the next head's warmup scores, so the
                # exp latency never surfaces at head boundaries.
                prev_rest = None

                def run_head(Ti, h):
                    nonlocal prev_rest
                    w, r = make_head(Ti, h, psC_sc, psC_acc)
                    if prev_rest is None:
                        w()
                    else:
                        prev_rest(w)
                    prev_rest = r

                for h in range(HG):
                    run_head(0, h)
                stageB1(NT - 1)
                stageB2(NT - 2, tp_pool=psC_sc)
                stageB2(NT - 1, tp_pool=psC_sc)
                bwork.release()
                bstream.release()
                wts.release()
                for Ti in (1, 2, 3):
                    for h in range(HG):
                        if h == 1:
                            for part in range(4):
                                queue_proj(Ti - 1, part)
                        run_head(Ti, h)
                prev_rest(None)
                for part in range(4):
                    queue_proj(3, part, split=(part >= 2))
                while proj_fill:
                    proj_fill.pop(0)()
                psC_acc.release()
                psC_sc.release()
                dout.release()
                cwork.release()



    nc.finalize()
    return nc


def _host_tables(q_norm_w, k_norm_w):
    """RoPE cos/sin tables in [t, d] layout with norm weights folded in."""
    half = HD // 2
    inv_freq = (
        1.0 / (ROPE_BASE ** (np.arange(0, half, dtype=np.float32) / half))
    ).astype(np.float32)
    ang = np.arange(T, dtype=np.float32)[:, None] * inv_freq[None, :]  # [T, half]
    cos = np.cos(ang).astype(np.float32)
    sin = np.sin(ang).astype(np.float32)
    cos2 = np.concatenate([cos, cos], axis=1)           # [T, 128]
    sin2 = np.concatenate([-sin, sin], axis=1)          # [T, 128]
    # q tables carry the softmax 1/sqrt(d) scale (rs_q rides in the rope
    # mults, rs_k in the exp scale)
    csq1 = cos2 * q_norm_w[None, :] * SCALE
    snq1 = sin2 * q_norm_w[None, :] * SCALE
    csq = np.ascontiguousarray(csq1, dtype=np.float32)  # [T, 128]
    snq = np.ascontiguousarray(snq1, dtype=np.float32)
    csk = (cos2 * k_norm_w[None, :]).astype(np.float32)
    snk = (sin2 * k_norm_w[None, :]).astype(np.float32)
    return np.ascontiguousarray(
        np.concatenate([csq, snq, csk, snk], axis=1)
    )


def _host_masks():
    idx = np.arange(P)
    stair = (idx[None, :] // BLOCK >= idx[:, None] // BLOCK).astype(NPBF16)
    return np.ascontiguousarray(stair)


def _host_x(xb):
    """[T, C] -> [NT, 128c, NC16, 128t] blocked-transposed bf16 layout."""
    return np.ascontiguousarray(
        xb.reshape(NT, P, NC16, P).transpose(0, 3, 2, 1).astype(NPBF16)
    )


_nc_cache = None


def kernel(x, W_attn, W_proj, q_norm_w, k_norm_w):
    global _nc_cache
    x = np.asarray(x, dtype=np.float32)
    W_attn = np.asarray(W_attn, dtype=np.float32)
    W_proj = np.asarray(W_proj, dtype=np.float32)
    q_norm_w = np.asarray(q_norm_w, dtype=np.float32)
    k_norm_w = np.asarray(k_norm_w, dtype=np.float32)
    B = x.shape[0]

    tblarr = _host_tables(q_norm_w, k_norm_w)
    dm1 = _host_masks()
    xts = [_host_x(x[b]) for b in range(B)]

    in_maps = []
    for core in range(8):
        b, g = divmod(core, 4)
        wa_core = np.concatenate(
            [
                W_attn[:, g * QCOLS : (g + 1) * QCOLS],
                W_attn[:, C + g * HD : C + (g + 1) * HD],
                W_attn[:, C + N_KV * HD + g * HD : C + N_KV * HD + (g + 1) * HD],
            ],
            axis=1,
        )
        wp_core = W_proj[g * QCOLS : (g + 1) * QCOLS, :]
        in_maps.append(
            {
                "xt": xts[b],
                "wa": np.ascontiguousarray(wa_core.astype(NPBF16)),
                "wp": np.ascontiguousarray(wp_core.astype(NPBF16)),
                "tbl": tblarr,
                "dm1": dm1,
            }
        )

    if _nc_cache is None:
        _nc_cache = build_nc()
    res = run_bass_kernel_spmd(_nc_cache, in_maps, core_ids=list(range(8)))

    out = np.zeros((B, T, C), dtype=np.float32)
    for core in range(8):
        b = core // 4
        out[b] += res.results[core]["out"].astype(np.float32)
    return out

